# revision 12
# baseline (speedup 1.0000x reference)
"""Criss-cross height-attention kernel for TRN2, 8 NeuronCores.

Reference computation (per batch b, independent per width column w):
    q = Wq@x+bq  [64,H,W];  k = Wk@x+bk;  v = Wv@x+bv  [512,H,W]
    energy[w,i,j] = sum_c q[c,i,w] k[c,j,w]
    att = softmax_j(energy);  out = gamma * (v @ att^T) + x

Sharding: data-parallel over B (8 batches -> 8 cores), no collectives.

Per-core plan (C=512, H=W=96, HW=9216):
  - host folds gamma into Wv/bv, pre-transposes weights
  - x loaded once, cast to bf16 (4 tiles [128, 9216])
  - qk projection upfront: [128, 9216] bf16 (big matmuls; q rows 0:64, k 64:128)
  - per w: energy = q_w^T k_w (PE) -> exp+rowsum (ACT accum) -> recip (DVE)
    -> normalize (ACT scale) -> transpose att (PE) -> vT_w = x_w^T WvT (PE,
    bias via K=1 ones matmul) -> out_chunk = vT^T @ attNT (PE)
    -> residual add vs x (DVE) -> staged DMA out per w-block
  - softmax skips max-subtraction: energy ~ N(0,64); exp overflow needs
    |E|>88 = 11 sigma => never happens.
"""
import numpy as np

try:
    import concourse.bass as bass
except ImportError:
    import sys
    sys.path.insert(0, '/root/.axon_site/_ro/trn_rl_repo')
    import concourse.bass as bass
from concourse import bacc
import concourse.tile as tile
from concourse import masks, mybir
from concourse.bass_utils import run_bass_kernel_spmd

F32 = mybir.dt.float32
BF16 = mybir.dt.bfloat16
AF = mybir.ActivationFunctionType

B, C, C8, H, W = 8, 512, 64, 96, 96
HW = H * W
NKC = C // 128          # 4 contraction chunks
WB = 32                 # w-block size for output staging
NBLK = W // WB          # 6 blocks
WG = 4                  # w-group sharing one psum3 bank
NT = HW // 512          # 18 projection N-tiles


def build():
    nc = bacc.Bacc()
    # x is pre-cast to bf16 and pre-transposed to w-major (free = w*H + h)
    # on the host, so every per-w slice is contiguous on chip.
    x_ext = nc.declare_dram_parameter("x", [C, HW], BF16, isOutput=False)
    wqkT_ext = nc.declare_dram_parameter("wqkT", [C, 128], BF16, isOutput=False)
    bqk_ext = nc.declare_dram_parameter("bqk", [128, 1], F32, isOutput=False)
    wvT_ext = nc.declare_dram_parameter("wvT", [C, C], BF16, isOutput=False)
    bvg_ext = nc.declare_dram_parameter("bvg", [1, C], BF16, isOutput=False)
    out_ext = nc.declare_dram_parameter("out", [C, HW], F32, isOutput=True)

    with tile.TileContext(nc) as tc:
        with (
            tc.tile_pool(name="xpool", bufs=1) as xpool,
            tc.tile_pool(name="wpool", bufs=1) as wpool,
            tc.tile_pool(name="qkpool", bufs=1) as qkpool,
            tc.tile_pool(name="ostage", bufs=2) as ostage,
            tc.tile_pool(name="vsb", bufs=6) as vsb,
            tc.tile_pool(name="attsb", bufs=6) as attsb,
            tc.tile_pool(name="zsb", bufs=4) as zsb,
        ):
            # ---- weights / constants ----
            wqkT = []
            wvT = []
            for kc in range(NKC):
                t = wpool.tile([128, 128], BF16, tag=f"wqk{kc}")
                nc.sync.dma_start(out=t[:], in_=wqkT_ext[kc * 128:(kc + 1) * 128, :])
                wqkT.append(t)
                t2 = wpool.tile([128, C], BF16, tag=f"wv{kc}")
                nc.sync.dma_start(out=t2[:], in_=wvT_ext[kc * 128:(kc + 1) * 128, :])
                wvT.append(t2)
            bq_sb = wpool.tile([C8, 1], F32, tag="bq")
            nc.sync.dma_start(out=bq_sb[:], in_=bqk_ext[0:C8, :])
            bk_sb = wpool.tile([C8, 1], F32, tag="bk")
            nc.sync.dma_start(out=bk_sb[:], in_=bqk_ext[C8:128, :])
            bvg = wpool.tile([1, C], BF16, tag="bvg")
            nc.sync.dma_start(out=bvg[:], in_=bvg_ext[:])
            ident = wpool.tile([H, H], BF16, tag="ident")
            masks.make_identity(nc, ident[:])
            # persistent rotating tiles with a 97th row carrying the v-bias
            # trick: softmax rows sum to 1, so appending vT row96=bv and
            # attNT row96=1 folds "+bv" into the K=97 contraction for free.
            NROT = 8
            vts_all = []
            attnt_all = []
            for i in range(NROT):
                vt = wpool.tile([H + 1, C], BF16, tag=f"vtp{i}", name=f"vtp{i}")
                nc.vector.tensor_copy(vt[H:H + 1, :], bvg[:])
                vts_all.append(vt)
                at = wpool.tile([H + 1, H], BF16, tag=f"atp{i}", name=f"atp{i}")
                nc.vector.memset(at[H:H + 1, :], 1.0)
                attnt_all.append(at)

            # ---- x load (bf16, w-major), 4 chunks ----
            xb = []
            for kc in range(NKC):
                t = xpool.tile([128, HW], BF16, tag=f"x{kc}", name=f"x{kc}")
                hw2 = HW // 2
                for half in range(2):
                    nc.sync.dma_start(
                        out=t[:, half * hw2:(half + 1) * hw2],
                        in_=x_ext[kc * 128:(kc + 1) * 128,
                                  half * hw2:(half + 1) * hw2])
                xb.append(t)
            # [p, w, h] views: contiguous h runs per w
            xv = [t[:].rearrange("p (w h) -> p w h", h=H) for t in xb]
            # [p, h, w] views for residual (h stride 1, w stride H)
            xhw = [t[:].rearrange("p (w h) -> p h w", h=H) for t in xb]

            # ---- q/k projections: [64, HW] bf16 each (base partition 0) ----
            q_sb = qkpool.tile([C8, HW], BF16, tag="q")
            k_sb = qkpool.tile([C8, HW], BF16, tag="k")
            with tc.tile_pool(name="qk_ps", bufs=2, space="PSUM") as qk_ps:
                for nt in range(NT):
                    sl = slice(nt * 512, (nt + 1) * 512)
                    psq = qk_ps.tile([C8, 512], F32, tag="qps")
                    psk = qk_ps.tile([C8, 512], F32, tag="kps")
                    for kc in range(NKC):
                        nc.tensor.matmul(
                            psq[:], wqkT[kc][:, 0:C8], xb[kc][:, sl],
                            start=(kc == 0), stop=(kc == NKC - 1),
                        )
                    for kc in range(NKC):
                        nc.tensor.matmul(
                            psk[:], wqkT[kc][:, C8:128], xb[kc][:, sl],
                            start=(kc == 0), stop=(kc == NKC - 1),
                        )
                    nc.scalar.activation(q_sb[:, sl], psq[:], AF.Identity,
                                         bias=bq_sb[:])
                    nc.scalar.activation(k_sb[:, sl], psk[:], AF.Identity,
                                         bias=bk_sb[:])
            qv = q_sb[:].rearrange("p (w h) -> p w h", h=H)
            kv = k_sb[:].rearrange("p (w h) -> p w h", h=H)

            # ---- main loop over w ----
            with (
                tc.tile_pool(name="e_ps", bufs=2, space="PSUM") as e_ps,
                tc.tile_pool(name="t_ps", bufs=2, space="PSUM") as t_ps,
                tc.tile_pool(name="v_ps", bufs=2, space="PSUM") as v_ps,
                tc.tile_pool(name="o_ps", bufs=2, space="PSUM") as o_ps,
            ):
                rot = 0
                for blk in range(NBLK):
                    ost = [ostage.tile([128, H * WB], BF16, tag=f"ost{kc}",
                                       name=f"ost{kc}") for kc in range(NKC)]
                    osthw = [t[:].rearrange("p (h w) -> p h w", h=H) for t in ost]
                    for grp in range(WB // WG):
                        vts = []
                        atts = []
                        attns = []

                        def do_transpose(wi):
                            # transpose attN -> attNT (rows 0:96 of the
                            # persistent K=97 tile; row 96 stays 1.0)
                            atps = t_ps.tile([H, H], BF16, tag="attps",
                                             name="atps")
                            nc.tensor.transpose(atps[:], attns[wi][:],
                                                ident[:])
                            nc.scalar.copy(atts[wi][0:H, :], atps[:])

                        for wi in range(WG):
                            w = blk * WB + grp * WG + wi
                            vt = vts_all[rot]
                            at = attnt_all[rot]
                            rot = (rot + 1) % NROT
                            vts.append(vt)
                            atts.append(at)
                            qs = qv[:, w, :]       # [64, 96] strided
                            ks = kv[:, w, :]
                            # energy[i,j] PSUM f32
                            eps = e_ps.tile([H, H], F32, tag="energy")
                            nc.tensor.matmul(eps[:], qs, ks, start=True,
                                             stop=True)
                            # attU = exp(energy) bf16 + row sums (ACT only
                            # runs Exp in steady state: no LUT reloads)
                            attu = attsb.tile([H, H], BF16, tag="attu")
                            zrow = zsb.tile([H, 1], F32, tag="z")
                            nc.scalar.activation(
                                attu[:], eps[:], AF.Exp, accum_out=zrow[:],
                            )
                            rz = zsb.tile([H, 1], F32, tag="rz")
                            nc.vector.reciprocal(rz[:], zrow[:])
                            # attN = attU * (1/Z) per-partition scale (DVE)
                            attn = attsb.tile([H, H], BF16, tag="attn")
                            nc.vector.tensor_scalar_mul(attn[:], attu[:],
                                                        rz[:])
                            attns.append(attn)
                            # vT_w [96, 512] = sum_kc x_w(kc)^T @ WvT(kc)
                            vps = v_ps.tile([H, C], F32, tag="vps")
                            for kc in range(NKC):
                                nc.tensor.matmul(
                                    vps[:], xv[kc][:, w, :], wvT[kc][:],
                                    start=(kc == 0), stop=(kc == NKC - 1),
                                )
                            nc.vector.tensor_copy(vt[0:H, :], vps[:])
                            # delay transposes so the softmax chain resolves
                            # while PE chews the next w's matmuls
                            if wi >= 1:
                                do_transpose(wi - 1)
                        do_transpose(WG - 1)
                        # out chunks: psum3[c128, WG*96], then residual + stage
                        w0 = grp * WG   # group offset within block
                        for kc in range(NKC):
                            ops = o_ps.tile([128, WG * H], F32, tag="ops")
                            for wi in range(WG):
                                nc.tensor.matmul(
                                    ops[:, wi * H:(wi + 1) * H],
                                    vts[wi][:, kc * 128:(kc + 1) * 128],
                                    atts[wi][:],
                                    start=True, stop=True,
                                )
                            # iterate (w outer, h inner): psum + x inner-
                            # contiguous; only the staged output is strided
                            pss = ops[:].rearrange("p (w h) -> p w h", w=WG)
                            xs = xv[kc][:, blk * WB + w0: blk * WB + w0 + WG, :]
                            dst = osthw[kc][:, :, w0:w0 + WG].rearrange(
                                "p h w -> p w h")
                            nc.vector.tensor_add(dst, pss, xs)
                    # DMA block out (w-contiguous 128B runs); SWDGE on the
                    # otherwise-idle gpsimd queue upcasts bf16 -> f32 inline
                    for kc in range(NKC):
                        dram = out_ext[kc * 128:(kc + 1) * 128, :].rearrange(
                            "p (h w) -> p h w", h=H)[:, :, blk * WB:(blk + 1) * WB]
                        nc.gpsimd.dma_start(out=dram, in_=osthw[kc][:, :, :])
    nc.compile()
    return nc


_NC = None


def kernel(x, Wq, bq, Wk, bk, Wv, bv, gamma):
    global _NC
    if _NC is None:
        _NC = build()
    in_maps = prep_in_maps({"x": x, "Wq": Wq, "bq": bq, "Wk": Wk, "bk": bk,
                            "Wv": Wv, "bv": bv, "gamma": gamma})
    res = run_bass_kernel_spmd(_NC, in_maps, core_ids=list(range(B)))
    out = np.stack([res.results[i]["out"].reshape(C, H, W) for i in range(B)])
    return out.astype(np.float32)

def prep_in_maps(inputs):
    """Shard + pre-format inputs per core: x -> bf16 w-major [C, W*H]."""
    import ml_dtypes
    bf16 = ml_dtypes.bfloat16
    x = np.asarray(inputs["x"], dtype=np.float32)
    g = float(np.asarray(inputs["gamma"]).reshape(-1)[0])
    wqkT = np.ascontiguousarray(
        np.concatenate([np.asarray(inputs["Wq"]), np.asarray(inputs["Wk"])],
                       axis=0).T).astype(bf16)
    bqk = np.concatenate(
        [np.asarray(inputs["bq"]), np.asarray(inputs["bk"])]
    ).reshape(128, 1).astype(np.float32)
    wvT = np.ascontiguousarray(
        (np.asarray(inputs["Wv"]) * g).T).astype(bf16)
    bvg = (np.asarray(inputs["bv"]) * g).reshape(1, C).astype(bf16)
    # [B, C, H, W] -> per-core [C, W, H] (w-major) bf16
    xt = np.ascontiguousarray(x.transpose(0, 1, 3, 2)).astype(bf16)
    return [
        {"x": xt[i].reshape(C, HW),
         "wqkT": wqkT, "bqk": bqk, "wvT": wvT, "bvg": bvg}
        for i in range(B)
    ]


# revision 13
# speedup vs baseline: 1.2684x; 1.2684x over previous
"""Criss-cross height-attention kernel for TRN2, 8 NeuronCores.

Reference computation (per batch b, independent per width column w):
    q = Wq@x+bq  [64,H,W];  k = Wk@x+bk;  v = Wv@x+bv  [512,H,W]
    energy[w,i,j] = sum_c q[c,i,w] k[c,j,w]
    att = softmax_j(energy);  out = gamma * (v @ att^T) + x

Sharding: data-parallel over B (8 batches -> 8 cores), no collectives.

Per-core plan (C=512, H=W=96, HW=9216):
  - host folds gamma into Wv/bv, pre-transposes weights
  - x loaded once, cast to bf16 (4 tiles [128, 9216])
  - qk projection upfront: [128, 9216] bf16 (big matmuls; q rows 0:64, k 64:128)
  - per w: energy = q_w^T k_w (PE) -> exp+rowsum (ACT accum) -> recip (DVE)
    -> normalize (ACT scale) -> transpose att (PE) -> vT_w = x_w^T WvT (PE,
    bias via K=1 ones matmul) -> out_chunk = vT^T @ attNT (PE)
    -> residual add vs x (DVE) -> staged DMA out per w-block
  - softmax skips max-subtraction: energy ~ N(0,64); exp overflow needs
    |E|>88 = 11 sigma => never happens.
"""
import numpy as np

try:
    import concourse.bass as bass
except ImportError:
    import sys
    sys.path.insert(0, '/root/.axon_site/_ro/trn_rl_repo')
    import concourse.bass as bass
from concourse import bacc
import concourse.tile as tile
from concourse import masks, mybir
from concourse.bass_utils import run_bass_kernel_spmd

F32 = mybir.dt.float32
BF16 = mybir.dt.bfloat16
AF = mybir.ActivationFunctionType

B, C, C8, H, W = 8, 512, 64, 96, 96
HW = H * W
NKC = C // 128          # 4 contraction chunks
WB = 32                 # w-block size for output staging
NBLK = W // WB          # 6 blocks
WG = 4                  # w-group sharing one psum3 bank
NT = HW // 512          # 18 projection N-tiles


def build():
    nc = bacc.Bacc()
    # x is pre-cast to bf16 and pre-transposed to w-major (free = w*H + h)
    # on the host, so every per-w slice is contiguous on chip.
    x_ext = nc.declare_dram_parameter("x", [C, HW], BF16, isOutput=False)
    wqkT_ext = nc.declare_dram_parameter("wqkT", [C, 128], BF16, isOutput=False)
    bqk_ext = nc.declare_dram_parameter("bqk", [128, 1], F32, isOutput=False)
    wvT_ext = nc.declare_dram_parameter("wvT", [C, C], BF16, isOutput=False)
    bvg_ext = nc.declare_dram_parameter("bvg", [1, C], BF16, isOutput=False)
    out_ext = nc.declare_dram_parameter("out", [C, HW], F32, isOutput=True)

    with tile.TileContext(nc) as tc:
        with (
            tc.tile_pool(name="xpool", bufs=1) as xpool,
            tc.tile_pool(name="wpool", bufs=1) as wpool,
            tc.tile_pool(name="qkpool", bufs=1) as qkpool,
            tc.tile_pool(name="ostage", bufs=2) as ostage,
            tc.tile_pool(name="vsb", bufs=6) as vsb,
            tc.tile_pool(name="attsb", bufs=6) as attsb,
            tc.tile_pool(name="zsb", bufs=4) as zsb,
        ):
            # ---- weights / constants ----
            wqkT = []
            wvT = []
            for kc in range(NKC):
                t = wpool.tile([128, 128], BF16, tag=f"wqk{kc}")
                nc.sync.dma_start(out=t[:], in_=wqkT_ext[kc * 128:(kc + 1) * 128, :])
                wqkT.append(t)
                t2 = wpool.tile([128, C], BF16, tag=f"wv{kc}")
                nc.sync.dma_start(out=t2[:], in_=wvT_ext[kc * 128:(kc + 1) * 128, :])
                wvT.append(t2)
            bq_sb = wpool.tile([C8, 1], F32, tag="bq")
            nc.sync.dma_start(out=bq_sb[:], in_=bqk_ext[0:C8, :])
            bk_sb = wpool.tile([C8, 1], F32, tag="bk")
            nc.sync.dma_start(out=bk_sb[:], in_=bqk_ext[C8:128, :])
            bvg = wpool.tile([1, C], BF16, tag="bvg")
            nc.sync.dma_start(out=bvg[:], in_=bvg_ext[:])
            ident = wpool.tile([H, H], BF16, tag="ident")
            masks.make_identity(nc, ident[:])
            # persistent rotating tiles with a 97th row carrying the v-bias
            # trick: softmax rows sum to 1, so appending vT row96=bv and
            # attNT row96=1 folds "+bv" into the K=97 contraction for free.
            NROT = 8
            vts_all = []
            attnt_all = []
            for i in range(NROT):
                vt = wpool.tile([H + 1, C], BF16, tag=f"vtp{i}", name=f"vtp{i}")
                nc.vector.tensor_copy(vt[H:H + 1, :], bvg[:])
                vts_all.append(vt)
                at = wpool.tile([H + 1, H], BF16, tag=f"atp{i}", name=f"atp{i}")
                nc.vector.memset(at[H:H + 1, :], 1.0)
                attnt_all.append(at)

            # ---- x load (bf16, w-major), 4 chunks ----
            xb = []
            for kc in range(NKC):
                t = xpool.tile([128, HW], BF16, tag=f"x{kc}", name=f"x{kc}")
                hw2 = HW // 2
                for half in range(2):
                    nc.sync.dma_start(
                        out=t[:, half * hw2:(half + 1) * hw2],
                        in_=x_ext[kc * 128:(kc + 1) * 128,
                                  half * hw2:(half + 1) * hw2])
                xb.append(t)
            # [p, w, h] views: contiguous h runs per w
            xv = [t[:].rearrange("p (w h) -> p w h", h=H) for t in xb]
            # [p, h, w] views for residual (h stride 1, w stride H)
            xhw = [t[:].rearrange("p (w h) -> p h w", h=H) for t in xb]

            # ---- q/k projections: [64, HW] bf16 each (base partition 0) ----
            q_sb = qkpool.tile([C8, HW], BF16, tag="q")
            k_sb = qkpool.tile([C8, HW], BF16, tag="k")
            with tc.tile_pool(name="qk_ps", bufs=2, space="PSUM") as qk_ps:
                for nt in range(NT):
                    sl = slice(nt * 512, (nt + 1) * 512)
                    psq = qk_ps.tile([C8, 512], F32, tag="qps")
                    psk = qk_ps.tile([C8, 512], F32, tag="kps")
                    for kc in range(NKC):
                        nc.tensor.matmul(
                            psq[:], wqkT[kc][:, 0:C8], xb[kc][:, sl],
                            start=(kc == 0), stop=(kc == NKC - 1),
                        )
                    for kc in range(NKC):
                        nc.tensor.matmul(
                            psk[:], wqkT[kc][:, C8:128], xb[kc][:, sl],
                            start=(kc == 0), stop=(kc == NKC - 1),
                        )
                    nc.scalar.activation(q_sb[:, sl], psq[:], AF.Identity,
                                         bias=bq_sb[:])
                    nc.scalar.activation(k_sb[:, sl], psk[:], AF.Identity,
                                         bias=bk_sb[:])
            qv = q_sb[:].rearrange("p (w h) -> p w h", h=H)
            kv = k_sb[:].rearrange("p (w h) -> p w h", h=H)

            # ---- main loop over w ----
            with (
                tc.tile_pool(name="e_ps", bufs=2, space="PSUM") as e_ps,
                tc.tile_pool(name="t_ps", bufs=2, space="PSUM") as t_ps,
                tc.tile_pool(name="v_ps", bufs=2, space="PSUM") as v_ps,
                tc.tile_pool(name="o_ps", bufs=2, space="PSUM") as o_ps,
            ):
                rot = 0
                for blk in range(NBLK):
                    ost = [ostage.tile([128, H * WB], BF16, tag=f"ost{kc}",
                                       name=f"ost{kc}") for kc in range(NKC)]
                    osthw = [t[:].rearrange("p (h w) -> p h w", h=H) for t in ost]
                    for grp in range(WB // WG):
                        vts = []
                        atts = []
                        attns = []

                        def do_transpose(wi):
                            # transpose attN -> attNT (rows 0:96 of the
                            # persistent K=97 tile; row 96 stays 1.0)
                            atps = t_ps.tile([H, H], BF16, tag="attps",
                                             name="atps")
                            nc.tensor.transpose(atps[:], attns[wi][:],
                                                ident[:])
                            nc.scalar.copy(atts[wi][0:H, :], atps[:])

                        for wi in range(WG):
                            w = blk * WB + grp * WG + wi
                            vt = vts_all[rot]
                            at = attnt_all[rot]
                            rot = (rot + 1) % NROT
                            vts.append(vt)
                            atts.append(at)
                            qs = qv[:, w, :]       # [64, 96] strided
                            ks = kv[:, w, :]
                            # energy[i,j] PSUM f32
                            eps = e_ps.tile([H, H], F32, tag="energy")
                            nc.tensor.matmul(eps[:], qs, ks, start=True,
                                             stop=True)
                            # attU = exp(energy) bf16 + row sums (ACT only
                            # runs Exp in steady state: no LUT reloads)
                            attu = attsb.tile([H, H], BF16, tag="attu")
                            zrow = zsb.tile([H, 1], F32, tag="z")
                            nc.scalar.activation(
                                attu[:], eps[:], AF.Exp, accum_out=zrow[:],
                            )
                            rz = zsb.tile([H, 1], F32, tag="rz")
                            nc.vector.reciprocal(rz[:], zrow[:])
                            # attN = attU * (1/Z) per-partition scale (DVE)
                            attn = attsb.tile([H, H], BF16, tag="attn")
                            nc.vector.tensor_scalar_mul(attn[:], attu[:],
                                                        rz[:])
                            attns.append(attn)
                            # vT_w [96, 512] = sum_kc x_w(kc)^T @ WvT(kc)
                            vps = v_ps.tile([H, C], F32, tag="vps")
                            for kc in range(NKC):
                                nc.tensor.matmul(
                                    vps[:], xv[kc][:, w, :], wvT[kc][:],
                                    start=(kc == 0), stop=(kc == NKC - 1),
                                )
                            nc.vector.tensor_copy(vt[0:H, :], vps[:])
                            # delay transposes so the softmax chain resolves
                            # while PE chews the next w's matmuls
                            if wi >= 1:
                                do_transpose(wi - 1)
                        do_transpose(WG - 1)
                        # out chunks: psum3[c128, WG*96], then residual + stage
                        w0 = grp * WG   # group offset within block
                        for kc in range(NKC):
                            ops = o_ps.tile([128, WG * H], F32, tag="ops")
                            for wi in range(WG):
                                nc.tensor.matmul(
                                    ops[:, wi * H:(wi + 1) * H],
                                    vts[wi][:, kc * 128:(kc + 1) * 128],
                                    atts[wi][:],
                                    start=True, stop=True,
                                )
                            # iterate (h outer, w inner): unit-stride dst
                            # (strided reads beat strided writes on DVE)
                            pss = ops[:].rearrange("p (w h) -> p h w", w=WG)
                            xs = xhw[kc][:, :, blk * WB + w0: blk * WB + w0 + WG]
                            dst = osthw[kc][:, :, w0:w0 + WG]
                            nc.vector.tensor_add(dst, pss, xs)
                    # DMA block out (w-contiguous 128B runs); SWDGE on the
                    # otherwise-idle gpsimd queue upcasts bf16 -> f32 inline
                    for kc in range(NKC):
                        dram = out_ext[kc * 128:(kc + 1) * 128, :].rearrange(
                            "p (h w) -> p h w", h=H)[:, :, blk * WB:(blk + 1) * WB]
                        nc.gpsimd.dma_start(out=dram, in_=osthw[kc][:, :, :])
    nc.compile()
    return nc


_NC = None


def kernel(x, Wq, bq, Wk, bk, Wv, bv, gamma):
    global _NC
    if _NC is None:
        _NC = build()
    in_maps = prep_in_maps({"x": x, "Wq": Wq, "bq": bq, "Wk": Wk, "bk": bk,
                            "Wv": Wv, "bv": bv, "gamma": gamma})
    res = run_bass_kernel_spmd(_NC, in_maps, core_ids=list(range(B)))
    out = np.stack([res.results[i]["out"].reshape(C, H, W) for i in range(B)])
    return out.astype(np.float32)

def prep_in_maps(inputs):
    """Shard + pre-format inputs per core: x -> bf16 w-major [C, W*H]."""
    import ml_dtypes
    bf16 = ml_dtypes.bfloat16
    x = np.asarray(inputs["x"], dtype=np.float32)
    g = float(np.asarray(inputs["gamma"]).reshape(-1)[0])
    wqkT = np.ascontiguousarray(
        np.concatenate([np.asarray(inputs["Wq"]), np.asarray(inputs["Wk"])],
                       axis=0).T).astype(bf16)
    bqk = np.concatenate(
        [np.asarray(inputs["bq"]), np.asarray(inputs["bk"])]
    ).reshape(128, 1).astype(np.float32)
    wvT = np.ascontiguousarray(
        (np.asarray(inputs["Wv"]) * g).T).astype(bf16)
    bvg = (np.asarray(inputs["bv"]) * g).reshape(1, C).astype(bf16)
    # [B, C, H, W] -> per-core [C, W, H] (w-major) bf16
    xt = np.ascontiguousarray(x.transpose(0, 1, 3, 2)).astype(bf16)
    return [
        {"x": xt[i].reshape(C, HW),
         "wqkT": wqkT, "bqk": bqk, "wvT": wvT, "bvg": bvg}
        for i in range(B)
    ]


# revision 14
# speedup vs baseline: 1.8520x; 1.4601x over previous
"""Criss-cross height-attention kernel for TRN2, 8 NeuronCores.

Reference computation (per batch b, independent per width column w):
    q = Wq@x+bq  [64,H,W];  k = Wk@x+bk;  v = Wv@x+bv  [512,H,W]
    energy[w,i,j] = sum_c q[c,i,w] k[c,j,w]
    att = softmax_j(energy);  out = gamma * (v @ att^T) + x

Sharding: data-parallel over B (8 batches -> 8 cores), no collectives.

Per-core plan (C=512, H=W=96, HW=9216):
  - host folds gamma into Wv/bv, pre-transposes weights
  - x loaded once, cast to bf16 (4 tiles [128, 9216])
  - qk projection upfront: [128, 9216] bf16 (big matmuls; q rows 0:64, k 64:128)
  - per w: energy = q_w^T k_w (PE) -> exp+rowsum (ACT accum) -> recip (DVE)
    -> normalize (ACT scale) -> transpose att (PE) -> vT_w = x_w^T WvT (PE,
    bias via K=1 ones matmul) -> out_chunk = vT^T @ attNT (PE)
    -> residual add vs x (DVE) -> staged DMA out per w-block
  - softmax skips max-subtraction: energy ~ N(0,64); exp overflow needs
    |E|>88 = 11 sigma => never happens.
"""
import numpy as np

try:
    import concourse.bass as bass
except ImportError:
    import sys
    sys.path.insert(0, '/root/.axon_site/_ro/trn_rl_repo')
    import concourse.bass as bass
from concourse import bacc
import concourse.tile as tile
from concourse import masks, mybir
from concourse.bass_utils import run_bass_kernel_spmd

F32 = mybir.dt.float32
BF16 = mybir.dt.bfloat16
AF = mybir.ActivationFunctionType

B, C, C8, H, W = 8, 512, 64, 96, 96
HW = H * W
NKC = C // 128          # 4 contraction chunks
WB = 32                 # w-block size for output staging
NBLK = W // WB          # 6 blocks
WG = 4                  # w-group sharing one psum3 bank
NT = HW // 512          # 18 projection N-tiles


def build():
    nc = bacc.Bacc()
    # x is pre-cast to bf16 and pre-transposed to w-major (free = w*H + h)
    # on the host, so every per-w slice is contiguous on chip.
    x_ext = nc.declare_dram_parameter("x", [C, HW], BF16, isOutput=False)
    wqkT_ext = nc.declare_dram_parameter("wqkT", [C, 128], BF16, isOutput=False)
    bqk_ext = nc.declare_dram_parameter("bqk", [128, 1], F32, isOutput=False)
    wvT_ext = nc.declare_dram_parameter("wvT", [C, C], BF16, isOutput=False)
    bvg_ext = nc.declare_dram_parameter("bvg", [1, C], BF16, isOutput=False)
    # out is written w-major [C, W*H]; the host transposes back to [C, H, W]
    out_ext = nc.declare_dram_parameter("out", [C, HW], F32, isOutput=True)

    with tile.TileContext(nc) as tc:
        with (
            tc.tile_pool(name="xpool", bufs=1) as xpool,
            tc.tile_pool(name="wpool", bufs=1) as wpool,
            tc.tile_pool(name="qkpool", bufs=1) as qkpool,
            tc.tile_pool(name="ostage", bufs=2) as ostage,
            tc.tile_pool(name="vsb", bufs=6) as vsb,
            tc.tile_pool(name="attsb", bufs=6) as attsb,
            tc.tile_pool(name="zsb", bufs=4) as zsb,
        ):
            # ---- weights / constants ----
            wqkT = []
            wvT = []
            for kc in range(NKC):
                t = wpool.tile([128, 128], BF16, tag=f"wqk{kc}")
                nc.sync.dma_start(out=t[:], in_=wqkT_ext[kc * 128:(kc + 1) * 128, :])
                wqkT.append(t)
                t2 = wpool.tile([128, C], BF16, tag=f"wv{kc}")
                nc.sync.dma_start(out=t2[:], in_=wvT_ext[kc * 128:(kc + 1) * 128, :])
                wvT.append(t2)
            bq_sb = wpool.tile([C8, 1], F32, tag="bq")
            nc.sync.dma_start(out=bq_sb[:], in_=bqk_ext[0:C8, :])
            bk_sb = wpool.tile([C8, 1], F32, tag="bk")
            nc.sync.dma_start(out=bk_sb[:], in_=bqk_ext[C8:128, :])
            bvg = wpool.tile([1, C], BF16, tag="bvg")
            nc.sync.dma_start(out=bvg[:], in_=bvg_ext[:])
            ident = wpool.tile([H, H], BF16, tag="ident")
            masks.make_identity(nc, ident[:])
            # persistent rotating tiles with a 97th row carrying the v-bias
            # trick: softmax rows sum to 1, so appending vT row96=bv and
            # attNT row96=1 folds "+bv" into the K=97 contraction for free.
            NROT = 8
            vts_all = []
            attnt_all = []
            for i in range(NROT):
                vt = wpool.tile([H + 1, C], BF16, tag=f"vtp{i}", name=f"vtp{i}")
                nc.vector.tensor_copy(vt[H:H + 1, :], bvg[:])
                vts_all.append(vt)
                at = wpool.tile([H + 1, H], BF16, tag=f"atp{i}", name=f"atp{i}")
                nc.vector.memset(at[H:H + 1, :], 1.0)
                attnt_all.append(at)

            # ---- x load (bf16, w-major), 4 chunks ----
            xb = []
            for kc in range(NKC):
                t = xpool.tile([128, HW], BF16, tag=f"x{kc}", name=f"x{kc}")
                hw2 = HW // 2
                for half in range(2):
                    nc.sync.dma_start(
                        out=t[:, half * hw2:(half + 1) * hw2],
                        in_=x_ext[kc * 128:(kc + 1) * 128,
                                  half * hw2:(half + 1) * hw2])
                xb.append(t)
            # [p, w, h] views: contiguous h runs per w
            xv = [t[:].rearrange("p (w h) -> p w h", h=H) for t in xb]
            # [p, h, w] views for residual (h stride 1, w stride H)
            xhw = [t[:].rearrange("p (w h) -> p h w", h=H) for t in xb]

            # ---- q/k projections: [64, HW] bf16 each (base partition 0) ----
            q_sb = qkpool.tile([C8, HW], BF16, tag="q")
            k_sb = qkpool.tile([C8, HW], BF16, tag="k")
            with tc.tile_pool(name="qk_ps", bufs=2, space="PSUM") as qk_ps:
                for nt in range(NT):
                    sl = slice(nt * 512, (nt + 1) * 512)
                    psq = qk_ps.tile([C8, 512], F32, tag="qps")
                    psk = qk_ps.tile([C8, 512], F32, tag="kps")
                    for kc in range(NKC):
                        nc.tensor.matmul(
                            psq[:], wqkT[kc][:, 0:C8], xb[kc][:, sl],
                            start=(kc == 0), stop=(kc == NKC - 1),
                        )
                    for kc in range(NKC):
                        nc.tensor.matmul(
                            psk[:], wqkT[kc][:, C8:128], xb[kc][:, sl],
                            start=(kc == 0), stop=(kc == NKC - 1),
                        )
                    nc.scalar.activation(q_sb[:, sl], psq[:], AF.Identity,
                                         bias=bq_sb[:])
                    nc.scalar.activation(k_sb[:, sl], psk[:], AF.Identity,
                                         bias=bk_sb[:])
            qv = q_sb[:].rearrange("p (w h) -> p w h", h=H)
            kv = k_sb[:].rearrange("p (w h) -> p w h", h=H)

            # ---- main loop over w ----
            with (
                tc.tile_pool(name="e_ps", bufs=2, space="PSUM") as e_ps,
                tc.tile_pool(name="t_ps", bufs=2, space="PSUM") as t_ps,
                tc.tile_pool(name="v_ps", bufs=2, space="PSUM") as v_ps,
                tc.tile_pool(name="o_ps", bufs=2, space="PSUM") as o_ps,
            ):
                rot = 0
                for blk in range(NBLK):
                    ost = [ostage.tile([128, H * WB], BF16, tag=f"ost{kc}",
                                       name=f"ost{kc}") for kc in range(NKC)]
                    for grp in range(WB // WG):
                        vts = []
                        atts = []
                        attus = []
                        w0 = grp * WG   # group offset within block
                        z4 = zsb.tile([H, WG], F32, tag="z4")
                        rz4 = zsb.tile([H, WG], F32, tag="rz4")
                        for wi in range(WG):
                            w = blk * WB + w0 + wi
                            vt = vts_all[rot]
                            at = attnt_all[rot]
                            rot = (rot + 1) % NROT
                            vts.append(vt)
                            atts.append(at)
                            qs = qv[:, w, :]       # [64, 96] contiguous
                            ks = kv[:, w, :]
                            # energy[i,j] PSUM f32
                            eps = e_ps.tile([H, H], F32, tag="energy")
                            nc.tensor.matmul(eps[:], qs, ks, start=True,
                                             stop=True)
                            # attU = exp(energy) bf16 + row sums (ACT only
                            # runs Exp in steady state: no LUT reloads)
                            attu = attsb.tile([H, H], BF16, tag="attu")
                            nc.scalar.activation(
                                attu[:], eps[:], AF.Exp,
                                accum_out=z4[:, wi:wi + 1],
                            )
                            attus.append(attu)
                            # vT_w [96, 512] = sum_kc x_w(kc)^T @ WvT(kc)
                            vps = v_ps.tile([H, C], F32, tag="vps")
                            for kc in range(NKC):
                                nc.tensor.matmul(
                                    vps[:], xv[kc][:, w, :], wvT[kc][:],
                                    start=(kc == 0), stop=(kc == NKC - 1),
                                )
                            nc.vector.tensor_copy(vt[0:H, :], vps[:])
                        # one batched reciprocal for the group
                        nc.vector.reciprocal(rz4[:], z4[:])
                        for wi in range(WG):
                            # attN = attU * (1/Z); transpose -> K=97 tile
                            attn = attsb.tile([H, H], BF16, tag="attn")
                            nc.vector.tensor_scalar_mul(
                                attn[:], attus[wi][:], rz4[:, wi:wi + 1])
                            atps = t_ps.tile([H, H], BF16, tag="attps",
                                             name="atps")
                            nc.tensor.transpose(atps[:], attn[:], ident[:])
                            nc.scalar.copy(atts[wi][0:H, :], atps[:])
                        # out chunks: psum3[c128, WG*96] in (w,h) order,
                        # residual + stage all contiguous [128, WG*H]
                        for kc in range(NKC):
                            ops = o_ps.tile([128, WG * H], F32, tag="ops")
                            for wi in range(WG):
                                nc.tensor.matmul(
                                    ops[:, wi * H:(wi + 1) * H],
                                    vts[wi][:, kc * 128:(kc + 1) * 128],
                                    atts[wi][:],
                                    start=True, stop=True,
                                )
                            xs = xb[kc][:, (blk * WB + w0) * H:
                                        (blk * WB + w0 + WG) * H]
                            dst = ost[kc][:, w0 * H:(w0 + WG) * H]
                            nc.vector.tensor_add(dst, ops[:], xs)
                    # DMA block out: fully contiguous w-major [128, WB*H];
                    # SWDGE on the idle gpsimd queue upcasts bf16 -> f32
                    for kc in range(NKC):
                        dram = out_ext[kc * 128:(kc + 1) * 128,
                                       blk * WB * H:(blk + 1) * WB * H]
                        nc.gpsimd.dma_start(out=dram, in_=ost[kc][:])
    nc.compile()
    return nc


_NC = None


def kernel(x, Wq, bq, Wk, bk, Wv, bv, gamma):
    global _NC
    if _NC is None:
        _NC = build()
    in_maps = prep_in_maps({"x": x, "Wq": Wq, "bq": bq, "Wk": Wk, "bk": bk,
                            "Wv": Wv, "bv": bv, "gamma": gamma})
    res = run_bass_kernel_spmd(_NC, in_maps, core_ids=list(range(B)))
    out = np.stack([res.results[i]["out"].reshape(C, W, H) for i in range(B)])
    return np.ascontiguousarray(out.transpose(0, 1, 3, 2)).astype(np.float32)

def prep_in_maps(inputs):
    """Shard + pre-format inputs per core: x -> bf16 w-major [C, W*H]."""
    import ml_dtypes
    bf16 = ml_dtypes.bfloat16
    x = np.asarray(inputs["x"], dtype=np.float32)
    g = float(np.asarray(inputs["gamma"]).reshape(-1)[0])
    wqkT = np.ascontiguousarray(
        np.concatenate([np.asarray(inputs["Wq"]), np.asarray(inputs["Wk"])],
                       axis=0).T).astype(bf16)
    bqk = np.concatenate(
        [np.asarray(inputs["bq"]), np.asarray(inputs["bk"])]
    ).reshape(128, 1).astype(np.float32)
    wvT = np.ascontiguousarray(
        (np.asarray(inputs["Wv"]) * g).T).astype(bf16)
    bvg = (np.asarray(inputs["bv"]) * g).reshape(1, C).astype(bf16)
    # [B, C, H, W] -> per-core [C, W, H] (w-major) bf16
    xt = np.ascontiguousarray(x.transpose(0, 1, 3, 2)).astype(bf16)
    return [
        {"x": xt[i].reshape(C, HW),
         "wqkT": wqkT, "bqk": bqk, "wvT": wvT, "bvg": bvg}
        for i in range(B)
    ]


# revision 15
# speedup vs baseline: 2.0153x; 1.0881x over previous
"""Criss-cross height-attention kernel for TRN2, 8 NeuronCores.

Reference computation (per batch b, independent per width column w):
    q = Wq@x+bq  [64,H,W];  k = Wk@x+bk;  v = Wv@x+bv  [512,H,W]
    energy[w,i,j] = sum_c q[c,i,w] k[c,j,w]
    att = softmax_j(energy);  out = gamma * (v @ att^T) + x

Sharding: data-parallel over B (8 batches -> 8 cores), no collectives.

Per-core plan (C=512, H=W=96, HW=9216):
  - host folds gamma into Wv/bv, pre-transposes weights
  - x loaded once, cast to bf16 (4 tiles [128, 9216])
  - qk projection upfront: [128, 9216] bf16 (big matmuls; q rows 0:64, k 64:128)
  - per w: energy = q_w^T k_w (PE) -> exp+rowsum (ACT accum) -> recip (DVE)
    -> normalize (ACT scale) -> transpose att (PE) -> vT_w = x_w^T WvT (PE,
    bias via K=1 ones matmul) -> out_chunk = vT^T @ attNT (PE)
    -> residual add vs x (DVE) -> staged DMA out per w-block
  - softmax skips max-subtraction: energy ~ N(0,64); exp overflow needs
    |E|>88 = 11 sigma => never happens.
"""
import numpy as np

try:
    import concourse.bass as bass
except ImportError:
    import sys
    sys.path.insert(0, '/root/.axon_site/_ro/trn_rl_repo')
    import concourse.bass as bass
from concourse import bacc
import concourse.tile as tile
from concourse import masks, mybir
from concourse.bass_utils import run_bass_kernel_spmd

F32 = mybir.dt.float32
BF16 = mybir.dt.bfloat16
AF = mybir.ActivationFunctionType

B, C, C8, H, W = 8, 512, 64, 96, 96
HW = H * W
NKC = C // 128          # 4 contraction chunks
WB = 32                 # w-block size for output staging
NBLK = W // WB          # 6 blocks
WG = 4                  # w-group sharing one psum3 bank
NT = HW // 512          # 18 projection N-tiles


def build():
    nc = bacc.Bacc()
    # x is pre-cast to bf16 and pre-transposed to w-major (free = w*H + h)
    # on the host, so every per-w slice is contiguous on chip.
    x_ext = nc.declare_dram_parameter("x", [C, HW], BF16, isOutput=False)
    wqkT_ext = nc.declare_dram_parameter("wqkT", [C, 128], BF16, isOutput=False)
    bqk_ext = nc.declare_dram_parameter("bqk", [128, 1], F32, isOutput=False)
    wvT_ext = nc.declare_dram_parameter("wvT", [C, C], BF16, isOutput=False)
    bvg_ext = nc.declare_dram_parameter("bvg", [1, C], BF16, isOutput=False)
    # out is written w-major [C, W*H]; the host transposes back to [C, H, W]
    out_ext = nc.declare_dram_parameter("out", [C, HW], F32, isOutput=True)

    with tile.TileContext(nc) as tc:
        with (
            tc.tile_pool(name="xpool", bufs=1) as xpool,
            tc.tile_pool(name="wpool", bufs=1) as wpool,
            tc.tile_pool(name="qkpool", bufs=1) as qkpool,
            tc.tile_pool(name="ostage", bufs=2) as ostage,
            tc.tile_pool(name="vsb", bufs=6) as vsb,
            tc.tile_pool(name="attsb", bufs=6) as attsb,
            tc.tile_pool(name="zsb", bufs=4) as zsb,
        ):
            # ---- weights / constants ----
            wqkT = []
            wvT = []
            for kc in range(NKC):
                t = wpool.tile([128, 128], BF16, tag=f"wqk{kc}")
                nc.sync.dma_start(out=t[:], in_=wqkT_ext[kc * 128:(kc + 1) * 128, :])
                wqkT.append(t)
                t2 = wpool.tile([128, C], BF16, tag=f"wv{kc}")
                nc.sync.dma_start(out=t2[:], in_=wvT_ext[kc * 128:(kc + 1) * 128, :])
                wvT.append(t2)
            bq_sb = wpool.tile([C8, 1], F32, tag="bq")
            nc.sync.dma_start(out=bq_sb[:], in_=bqk_ext[0:C8, :])
            bk_sb = wpool.tile([C8, 1], F32, tag="bk")
            nc.sync.dma_start(out=bk_sb[:], in_=bqk_ext[C8:128, :])
            bvg = wpool.tile([1, C], BF16, tag="bvg")
            nc.sync.dma_start(out=bvg[:], in_=bvg_ext[:])
            ident = wpool.tile([H, H], BF16, tag="ident")
            masks.make_identity(nc, ident[:])
            # persistent rotating tiles with a 97th row carrying the v-bias
            # trick: softmax rows sum to 1, so appending vT row96=bv and
            # attNT row96=1 folds "+bv" into the K=97 contraction for free.
            NROT = 8
            vts_all = []
            attnt_all = []
            for i in range(NROT):
                vt = wpool.tile([H + 1, C], BF16, tag=f"vtp{i}", name=f"vtp{i}")
                nc.vector.tensor_copy(vt[H:H + 1, :], bvg[:])
                vts_all.append(vt)
                at = wpool.tile([H + 1, H], BF16, tag=f"atp{i}", name=f"atp{i}")
                nc.vector.memset(at[H:H + 1, :], 1.0)
                attnt_all.append(at)

            # ---- x load (bf16, w-major), 4 chunks ----
            xb = [xpool.tile([128, HW], BF16, tag=f"x{kc}", name=f"x{kc}")
                  for kc in range(NKC)]
            hw2 = HW // 2
            for half in range(2):
                for kc in range(NKC):
                    nc.sync.dma_start(
                        out=xb[kc][:, half * hw2:(half + 1) * hw2],
                        in_=x_ext[kc * 128:(kc + 1) * 128,
                                  half * hw2:(half + 1) * hw2])
            # [p, w, h] views: contiguous h runs per w
            xv = [t[:].rearrange("p (w h) -> p w h", h=H) for t in xb]
            # [p, h, w] views for residual (h stride 1, w stride H)
            xhw = [t[:].rearrange("p (w h) -> p h w", h=H) for t in xb]

            # ---- q/k projections: [64, HW] bf16 each (base partition 0) ----
            q_sb = qkpool.tile([C8, HW], BF16, tag="q")
            k_sb = qkpool.tile([C8, HW], BF16, tag="k")
            with tc.tile_pool(name="qk_ps", bufs=2, space="PSUM") as qk_ps:
                for nt in range(NT):
                    sl = slice(nt * 512, (nt + 1) * 512)
                    psq = qk_ps.tile([C8, 512], F32, tag="qps")
                    psk = qk_ps.tile([C8, 512], F32, tag="kps")
                    for kc in range(NKC):
                        nc.tensor.matmul(
                            psq[:], wqkT[kc][:, 0:C8], xb[kc][:, sl],
                            start=(kc == 0), stop=(kc == NKC - 1),
                        )
                    for kc in range(NKC):
                        nc.tensor.matmul(
                            psk[:], wqkT[kc][:, C8:128], xb[kc][:, sl],
                            start=(kc == 0), stop=(kc == NKC - 1),
                        )
                    nc.scalar.activation(q_sb[:, sl], psq[:], AF.Identity,
                                         bias=bq_sb[:])
                    nc.scalar.activation(k_sb[:, sl], psk[:], AF.Identity,
                                         bias=bk_sb[:])
            qv = q_sb[:].rearrange("p (w h) -> p w h", h=H)
            kv = k_sb[:].rearrange("p (w h) -> p w h", h=H)

            # ---- main loop over w ----
            with (
                tc.tile_pool(name="e_ps", bufs=2, space="PSUM") as e_ps,
                tc.tile_pool(name="t_ps", bufs=2, space="PSUM") as t_ps,
                tc.tile_pool(name="v_ps", bufs=2, space="PSUM") as v_ps,
                tc.tile_pool(name="o_ps", bufs=2, space="PSUM") as o_ps,
            ):
                rot = 0
                for blk in range(NBLK):
                    ost = [ostage.tile([128, H * WB], BF16, tag=f"ost{kc}",
                                       name=f"ost{kc}") for kc in range(NKC)]
                    for grp in range(WB // WG):
                        vts = []
                        atts = []
                        attus = []
                        w0 = grp * WG   # group offset within block
                        z4 = zsb.tile([H, WG], F32, tag="z4")
                        rz4 = zsb.tile([H, WG], F32, tag="rz4")
                        for wi in range(WG):
                            w = blk * WB + w0 + wi
                            vt = vts_all[rot]
                            at = attnt_all[rot]
                            rot = (rot + 1) % NROT
                            vts.append(vt)
                            atts.append(at)
                            qs = qv[:, w, :]       # [64, 96] contiguous
                            ks = kv[:, w, :]
                            # energy[i,j] PSUM f32
                            eps = e_ps.tile([H, H], F32, tag="energy")
                            nc.tensor.matmul(eps[:], qs, ks, start=True,
                                             stop=True)
                            # attU = exp(energy) bf16 + row sums (ACT only
                            # runs Exp in steady state: no LUT reloads)
                            attu = attsb.tile([H, H], BF16, tag="attu")
                            nc.scalar.activation(
                                attu[:], eps[:], AF.Exp,
                                accum_out=z4[:, wi:wi + 1],
                            )
                            attus.append(attu)
                            # vT_w [96, 512] = sum_kc x_w(kc)^T @ WvT(kc)
                            vps = v_ps.tile([H, C], F32, tag="vps")
                            for kc in range(NKC):
                                nc.tensor.matmul(
                                    vps[:], xv[kc][:, w, :], wvT[kc][:],
                                    start=(kc == 0), stop=(kc == NKC - 1),
                                )
                            nc.vector.tensor_copy(vt[0:H, :], vps[:])
                        # one batched reciprocal for the group
                        nc.vector.reciprocal(rz4[:], z4[:])
                        for wi in range(WG):
                            # attN = attU * (1/Z); transpose -> K=97 tile
                            attn = attsb.tile([H, H], BF16, tag="attn")
                            nc.vector.tensor_scalar_mul(
                                attn[:], attus[wi][:], rz4[:, wi:wi + 1])
                            atps = t_ps.tile([H, H], BF16, tag="attps",
                                             name="atps")
                            nc.tensor.transpose(atps[:], attn[:], ident[:])
                            nc.scalar.copy(atts[wi][0:H, :], atps[:])
                        # out chunks: psum3[c128, WG*96] in (w,h) order,
                        # residual + stage all contiguous [128, WG*H]
                        for kc in range(NKC):
                            ops = o_ps.tile([128, WG * H], F32, tag="ops")
                            for wi in range(WG):
                                nc.tensor.matmul(
                                    ops[:, wi * H:(wi + 1) * H],
                                    vts[wi][:, kc * 128:(kc + 1) * 128],
                                    atts[wi][:],
                                    start=True, stop=True,
                                )
                            xs = xb[kc][:, (blk * WB + w0) * H:
                                        (blk * WB + w0 + WG) * H]
                            dst = ost[kc][:, w0 * H:(w0 + WG) * H]
                            nc.vector.tensor_add(dst, ops[:], xs)
                    # DMA out per half-block: contiguous w-major; SWDGE on
                    # the idle gpsimd queue upcasts bf16 -> f32
                    for half in range(2):
                        wh2 = WB * H // 2
                        for kc in range(NKC):
                            dram = out_ext[kc * 128:(kc + 1) * 128,
                                           blk * WB * H + half * wh2:
                                           blk * WB * H + (half + 1) * wh2]
                            nc.gpsimd.dma_start(
                                out=dram,
                                in_=ost[kc][:, half * wh2:(half + 1) * wh2])
    nc.compile()
    return nc


_NC = None


def kernel(x, Wq, bq, Wk, bk, Wv, bv, gamma):
    global _NC
    if _NC is None:
        _NC = build()
    in_maps = prep_in_maps({"x": x, "Wq": Wq, "bq": bq, "Wk": Wk, "bk": bk,
                            "Wv": Wv, "bv": bv, "gamma": gamma})
    res = run_bass_kernel_spmd(_NC, in_maps, core_ids=list(range(B)))
    out = np.stack([res.results[i]["out"].reshape(C, W, H) for i in range(B)])
    return np.ascontiguousarray(out.transpose(0, 1, 3, 2)).astype(np.float32)

def prep_in_maps(inputs):
    """Shard + pre-format inputs per core: x -> bf16 w-major [C, W*H]."""
    import ml_dtypes
    bf16 = ml_dtypes.bfloat16
    x = np.asarray(inputs["x"], dtype=np.float32)
    g = float(np.asarray(inputs["gamma"]).reshape(-1)[0])
    wqkT = np.ascontiguousarray(
        np.concatenate([np.asarray(inputs["Wq"]), np.asarray(inputs["Wk"])],
                       axis=0).T).astype(bf16)
    bqk = np.concatenate(
        [np.asarray(inputs["bq"]), np.asarray(inputs["bk"])]
    ).reshape(128, 1).astype(np.float32)
    wvT = np.ascontiguousarray(
        (np.asarray(inputs["Wv"]) * g).T).astype(bf16)
    bvg = (np.asarray(inputs["bv"]) * g).reshape(1, C).astype(bf16)
    # [B, C, H, W] -> per-core [C, W, H] (w-major) bf16
    xt = np.ascontiguousarray(x.transpose(0, 1, 3, 2)).astype(bf16)
    return [
        {"x": xt[i].reshape(C, HW),
         "wqkT": wqkT, "bqk": bqk, "wvT": wvT, "bvg": bvg}
        for i in range(B)
    ]
